# revision 73
# baseline (speedup 1.0000x reference)
"""MixedExpertLayer Trainium2 kernel, v11: fully routed, globally balanced MoE.

ALL four experts are routed on the host. For each expert the tokens with
nonzero combined weight c_e = sum_k w_k*[idx_k==e] (~43.75% of tokens) are
gathered into a compacted feature-major stream and split EVENLY across the 8
cores (no core affinity), so each core processes ~900 tokens per expert.

MLP experts 0,1 (per core, per expert, windows of <=512 tokens):
  gate/up: psum[i-tile, tok] = sum_h W[h,i]^T x[h, tok]   (feature-major)
  A = silu(g)*u on ACT+DVE
  down (feature-major): psum[h-tile, tok] = sum_i Wd[i,h]^T A[i, tok]
  scaled by c_e via one DVE mul with a host-broadcast coefficient row.

Conv experts 2,3: the host gathers FOUR tap-shifted copies of the selected
tokens (x[t-3+j] for j=0..3, zero at sequence starts), so the depthwise conv
becomes 4 accumulated diag-matrix matmuls on the PE over compacted columns
(diag(w_j) built on-device from an identity via ACT per-partition scaling),
then ACT silu and a DVE mul by the broadcast c_e row.

All outputs are compacted feature-major [H, C_e]; the host transposes and
scatter-adds the four streams into the zero-initialized result (fp32).

Conv ops are fed through a drain queue that interleaves them between MLP
matmul groups so no in-order engine stream is head-of-line blocked.
"""

import numpy as np
import ml_dtypes

import concourse.bass as bass
import concourse.mybir as mybir
import concourse.tile as tile
from concourse.bass_utils import run_bass_kernel_spmd

B, S, H, I, KTOP, KC = 4, 4096, 1024, 2048, 2, 4
NCORES = 8
T = (B * S) // NCORES          # 2048 tokens per core
TCH = 512                      # token chunk (matmul N / PSUM bank)
HK = H // 128                  # 8 h-chunks
IK = I // 128                  # 16 i-chunks
BF16 = mybir.dt.bfloat16
F32 = mybir.dt.float32
AF = mybir.ActivationFunctionType
MUL = mybir.AluOpType.mult
ADD = mybir.AluOpType.add

# routing state set by build_in_maps: per-expert device capacities and
# per-(expert, core) global token lists
_ROUTE = {"C": [1024, 1024, 1024, 1024], "lists": None}


def legalize_waits(nc):
    """This walrus build encodes exactly one sync-wait per instruction
    (single NEURON_ISA_TPB_EVENTS slot); Tile emits up to 3 plus a multi-wait
    tail Drain. Split extra waits onto wait-only EventSemaphore carriers
    inserted immediately before the instruction (same engine, same position,
    so no reordering and no deadlock risk)."""
    f = nc.m.functions[0]
    for blk in f.blocks:
        new = []
        for ins in list(blk.instructions):
            si = ins.sync_info
            if si is not None and si.on_wait and len(si.on_wait) > 1:
                best, order = {}, []
                for w in si.on_wait:
                    k = (w.sync_type, w.id, w.wait_mode)
                    if k not in best:
                        best[k] = w
                        order.append(k)
                    elif (w.wait_value or 0) > (best[k].wait_value or 0):
                        best[k] = w
                waits = [best[k] for k in order]
                for j, w in enumerate(waits[:-1]):
                    ev = mybir.InstEventSemaphore(
                        name=f"{ins.name}-lw{j}", engine=ins.engine, ins=[], outs=[],
                    )
                    ev.sync_info = mybir.SyncInfo(on_wait=[w], on_update=[])
                    new.append(ev)
                si.on_wait = [waits[-1]]
                ins.sync_info = si
            new.append(ins)
        blk.instructions = new
    return nc


def _windows(Ce):
    """Chunk windows (w0, n) covering Ce tokens in <=TCH pieces."""
    w, out = 0, []
    while w < Ce:
        n = min(TCH, Ce - w)
        out.append((w, n))
        w += n
    return out


def build_nc():
    C0, C1, C2, C3 = _ROUTE["C"]
    nc = bass.Bass(num_devices=NCORES)
    xg0 = nc.dram_tensor("xg0", [128, HK, C0], BF16, kind="ExternalInput")
    xg1 = nc.dram_tensor("xg1", [128, HK, C1], BF16, kind="ExternalInput")
    # xc streams are pre-scaled by the conv tap weight on the host
    # (w[h,j] folded into the gathered activations), so the depthwise conv
    # reduces to three 2-operand adds per hk slab on the DVE
    xc0 = nc.dram_tensor("xc0", [128, HK, KC, C2], BF16, kind="ExternalInput")
    xc1 = nc.dram_tensor("xc1", [128, HK, KC, C3], BF16, kind="ExternalInput")
    wgr = nc.dram_tensor("wgr", [2, IK, 128, HK, 128], BF16, kind="ExternalInput")
    wur = nc.dram_tensor("wur", [2, IK, 128, HK, 128], BF16, kind="ExternalInput")
    wdr = nc.dram_tensor("wdr", [2, IK, 128, HK, 128], BF16, kind="ExternalInput")
    cgb0 = nc.dram_tensor("cgb0", [128, C0], BF16, kind="ExternalInput")
    cgb1 = nc.dram_tensor("cgb1", [128, C1], BF16, kind="ExternalInput")
    cgc0 = nc.dram_tensor("cgc0", [128, C2], BF16, kind="ExternalInput")
    cgc1 = nc.dram_tensor("cgc1", [128, C3], BF16, kind="ExternalInput")
    yf0 = nc.dram_tensor("yf0", [H, C0], BF16, kind="ExternalOutput")
    yf1 = nc.dram_tensor("yf1", [H, C1], BF16, kind="ExternalOutput")
    yc0 = nc.dram_tensor("yc0", [H, C2], BF16, kind="ExternalOutput")
    yc1 = nc.dram_tensor("yc1", [H, C3], BF16, kind="ExternalOutput")

    yf_t = [y.rearrange("(o p) t -> p o t", p=128) for y in (yf0, yf1)]
    yc_t = [y.rearrange("(o p) t -> p o t", p=128) for y in (yc0, yc1)]
    xg_d = [xg0, xg1]
    xc_d = [xc0, xc1]
    cgb_d = [cgb0, cgb1]
    cgc_d = [cgc0, cgc1]
    Cconv = [C2, C3]

    phases = [(0, w0, n) for (w0, n) in _windows(C0)] + \
             [(1, w0, n) for (w0, n) in _windows(C1)]

    with tile.TileContext(nc) as tc:
        with (
            tc.tile_pool(name="singles", bufs=1) as singles,
            tc.tile_pool(name="wpool", bufs=17) as wpool,
            tc.tile_pool(name="wdpool", bufs=17) as wdpool,
            tc.tile_pool(name="apool", bufs=2) as apool,
            tc.tile_pool(name="xcpool", bufs=3) as xcpool,
            tc.tile_pool(name="spool", bufs=3) as spool,
            tc.tile_pool(name="sgp", bufs=2) as sgpool,
            tc.tile_pool(name="ytp", bufs=3) as ytpool,
            tc.tile_pool(name="ps", bufs=3, space="PSUM") as ps,
            tc.tile_pool(name="pd", bufs=2, space="PSUM") as pd,
        ):
            # ---- PE clock warmup: dummy matmuls with no DMA deps run while
            # the first input tiles stream in, so the DVFS ramp (0.65 ->
            # 2.4GHz over ~3us of continuous execution) finishes before the
            # real matmuls start ----
            scr = singles.tile([128, TCH], BF16)
            nc.gpsimd.memset(scr, 0.0)
            pw = pd.tile([128, TCH], F32, tag="pd", name="pwarm")
            for r in range(32):
                nc.tensor.matmul(pw, scr[:, 0:128], scr,
                                 start=(r == 0), stop=(r == 31))

            # ---- phase-0-critical DMAs first: xg0, then small state ----
            xg_sb = [singles.tile([128, HK, Cx], BF16, name=f"xg{i}")
                     for i, Cx in enumerate((C0, C1))]

            def xg_load(e, split=0):
                Cx = xg_sb[e].shape[-1]
                for hk in range(HK):
                    if hk < split:
                        h2 = Cx // 2
                        nc.sync.dma_start(
                            xg_sb[e][:, hk, 0:h2], xg_d[e][:, hk, 0:h2])
                        nc.sync.dma_start(
                            xg_sb[e][:, hk, h2:Cx], xg_d[e][:, hk, h2:Cx])
                    else:
                        nc.sync.dma_start(xg_sb[e][:, hk], xg_d[e][:, hk])

            xg_load(0)
            cgb_sb = [singles.tile([128, Cx], BF16, name=f"cgb{i}")
                      for i, Cx in enumerate((C0, C1))]
            cgc_sb = [singles.tile([128, Cx], BF16, name=f"cgc{i}")
                      for i, Cx in enumerate((C2, C3))]

            # ---- conv experts: drain-queue units of (e, hk) ----
            xc_tiles = {}

            def xc_fetch(u):
                if u >= 2 * HK:
                    return
                e, hk = divmod(u, HK)
                xct = xcpool.tile([128, KC, Cconv[e]], BF16, tag="xc",
                                  name="xct")
                for j in range(KC):
                    nc.sync.dma_start(xct[:, j, :], xc_d[e][:, hk, j, :])
                xc_tiles[u] = xct

            CCMAX = max(C2, C3)

            def conv_unit(u):
                e, hk = divmod(u, HK)
                Cc = Cconv[e]
                xct = xc_tiles.pop(u)
                acc = [None]

                # adds/mul on DVE: Pool is idle but ~3.5x slower per op, and
                # its lag delays xct buffer recycling, which blocks the next
                # xc_fetch in the in-order SP queue and starves the weight
                # stream behind it (measured +26us)
                def add01():
                    acc[0] = spool.tile([128, CCMAX], BF16, tag="sc",
                                        name="cacc")
                    nc.vector.tensor_add(
                        acc[0][:, 0:Cc], xct[:, 0, :], xct[:, 1, :])

                def addj(j):
                    def op():
                        nc.vector.tensor_add(
                            acc[0][:, 0:Cc], acc[0][:, 0:Cc], xct[:, j, :])
                    return op

                def silu():
                    nc.scalar.activation(
                        out=acc[0][:, 0:Cc], in_=acc[0][:, 0:Cc], func=AF.Silu)

                def mul():
                    nc.vector.tensor_mul(
                        acc[0][:, 0:Cc], acc[0][:, 0:Cc], cgc_sb[e])

                def store():
                    nc.sync.dma_start(yc_t[e][:, hk, :], acc[0][:, 0:Cc])

                return [add01, addj(2), addj(3), silu, mul, store,
                        lambda: xc_fetch(u + 3)]

            # (gate_slot, fn): fn runs no earlier than slot gate_slot, so the
            # heavy conv-input DMAs don't steal queues from phase-0 weights
            pending = [
                (3, lambda: nc.sync.dma_start(cgb_sb[0], cgb_d[0][:])),
                (3, lambda: nc.sync.dma_start(cgb_sb[1], cgb_d[1][:])),
                (5, lambda: nc.sync.dma_start(cgc_sb[0], cgc_d[0][:])),
                (5, lambda: nc.sync.dma_start(cgc_sb[1], cgc_d[1][:])),
                (7, lambda: xc_fetch(0)),
                (9, lambda: xc_fetch(1)),
                (11, lambda: xc_fetch(2)),
            ]
            conv_left = list(range(2 * HK))

            # drain only in gate/up slots: ops drained during the down phase
            # land on the DVE right before the NEXT phase's A-muls (which
            # gate PSUM recycling) and stall the PE at phase boundaries
            nslots = sum(IK for _ in phases)
            dn = max(2, -(-(len(pending) + 2 * HK * 7) // max(nslots - 13, 1)))

            slot_idx = [0]

            def drain(k):
                slot_idx[0] += 1
                for _ in range(k):
                    if not pending:
                        if not conv_left or slot_idx[0] <= 13:
                            return
                        pending.extend(
                            (0, f) for f in conv_unit(conv_left.pop(0)))
                    gate, fn = pending[0]
                    if slot_idx[0] < gate:
                        return
                    pending.pop(0)
                    fn()

            wds_by_e = {}
            wgu_by_e = {}
            for pi, (e, w0, nw) in enumerate(phases):
                # ---- gate/up -> A (feature-major [I, nw]) ----
                a_sb = apool.tile([128, IK, TCH], BF16, tag="a")
                wload = e not in wgu_by_e
                if wload:
                    wgu_by_e[e] = []
                need_wd = e not in wds_by_e
                if need_wd:
                    wds_by_e[e] = []
                for i in range(IK):
                    if wload:
                        # gate/up tiles stay resident across the expert's
                        # windows - loaded only on its first phase
                        wgt = wpool.tile([128, HK, 128], BF16, tag="wg")
                        wut = wpool.tile([128, HK, 128], BF16, tag="wu")
                        wgu_by_e[e].append((wgt, wut))
                        if pi == 0 and i < 3:
                            # split the very first weight tiles across two
                            # queues each: their latency gates the PE rampup
                            nc.sync.dma_start(wgt[:, 0:4], wgr[e, i, :, 0:4])
                            nc.sync.dma_start(wgt[:, 4:8], wgr[e, i, :, 4:8])
                            nc.sync.dma_start(wut[:, 0:4], wur[e, i, :, 0:4])
                            nc.sync.dma_start(wut[:, 4:8], wur[e, i, :, 4:8])
                        else:
                            nc.sync.dma_start(wgt, wgr[e, i])
                            nc.sync.dma_start(wut, wur[e, i])
                    else:
                        wgt, wut = wgu_by_e[e][i]
                    psg = ps.tile([128, TCH], F32, tag="pg")
                    psu = ps.tile([128, TCH], F32, tag="pu")
                    for kc in range(HK):
                        nc.tensor.matmul(
                            psg[:, 0:nw], wgt[:, kc, :],
                            xg_sb[e][:, kc, w0 : w0 + nw],
                            start=(kc == 0), stop=(kc == HK - 1))
                    for kc in range(HK):
                        nc.tensor.matmul(
                            psu[:, 0:nw], wut[:, kc, :],
                            xg_sb[e][:, kc, w0 : w0 + nw],
                            start=(kc == 0), stop=(kc == HK - 1))
                    sg = sgpool.tile([128, TCH], F32, tag="sg")
                    nc.scalar.activation(
                        out=sg[:, 0:nw], in_=psg[:, 0:nw], func=AF.Silu)
                    nc.vector.tensor_mul(
                        a_sb[:, i, 0:nw], sg[:, 0:nw], psu[:, 0:nw])
                    # mid-phase prefetch of down weights (shared across the
                    # expert's windows) and the next xg stream, spread over
                    # several iterations - emitted as one burst, the serial
                    # SP descriptor-gen (~0.9us per dma_start) starves the
                    # next gate/up weight tiles and stalls the PE
                    if need_wd and 8 <= i < 12:
                        for kc in range(4 * (i - 8), 4 * (i - 7)):
                            wdt = wdpool.tile([128, HK, 128], BF16,
                                              tag="wd", name="wdt")
                            nc.sync.dma_start(wdt, wdr[e, kc])
                            wds_by_e[e].append(wdt)
                    if pi == 0 and i in (12, 13):
                        for hk in range(4 * (i - 12), 4 * (i - 11)):
                            nc.sync.dma_start(xg_sb[1][:, hk], xg_d[1][:, hk])
                    drain(dn)

                # ---- down, feature-major: psum[h-tile, tok] ----
                wds = wds_by_e[e]
                for hb in range(HK):
                    psd = pd.tile([128, TCH], F32, tag="pd")
                    for kc in range(IK):
                        nc.tensor.matmul(
                            psd[:, 0:nw], wds[kc][:, hb, :],
                            a_sb[:, kc, 0:nw],
                            start=(kc == 0), stop=(kc == IK - 1))
                    yt = ytpool.tile([128, TCH], BF16, tag="yt")
                    nc.vector.tensor_mul(
                        yt[:, 0:nw], psd[:, 0:nw],
                        cgb_sb[e][:, w0 : w0 + nw])
                    if pi == len(phases) - 1:
                        # last phase: split the store for lower tail latency
                        h2 = nw // 2
                        nc.sync.dma_start(
                            yf_t[e][:, hb, w0 : w0 + h2], yt[:, 0:h2])
                        nc.sync.dma_start(
                            yf_t[e][:, hb, w0 + h2 : w0 + nw], yt[:, h2:nw])
                    else:
                        nc.sync.dma_start(
                            yf_t[e][:, hb, w0 : w0 + nw], yt[:, 0:nw])
            # flush any remaining conv work
            while pending or conv_left:
                drain(16)
    return legalize_waits(nc)


def _bf16(a):
    return np.asarray(a).astype(ml_dtypes.bfloat16)


def build_in_maps(x, top_k_indices, norm_weights, mlp_gate, mlp_up, mlp_down, conv_w):
    NT = B * S
    xflat = np.asarray(x, dtype=np.float32).reshape(NT, H)
    xflat_b = _bf16(xflat)
    idxflat = np.asarray(top_k_indices).reshape(NT, KTOP)
    nwflat = np.asarray(norm_weights, dtype=np.float32).reshape(NT, KTOP)

    # combined per-expert coefficients, global
    ce = np.zeros((NT, 4), dtype=np.float32)
    rows = np.arange(NT)
    for k in range(KTOP):
        np.add.at(ce, (rows, idxflat[:, k]), nwflat[:, k])

    # globally balanced routing: split every expert's token list evenly
    lists, Cs = [], []
    for e in range(4):
        glst = np.nonzero(ce[:, e] != 0.0)[0]
        parts = np.array_split(glst, NCORES)
        lists.append(parts)
        Cs.append(max(1, max(len(p) for p in parts)))
    _ROUTE["C"] = Cs
    _ROUTE["lists"] = lists

    # weights, repacked so every DMA tile is contiguous per partition
    wgr = np.ascontiguousarray(
        _bf16(mlp_gate).reshape(2, HK, 128, IK, 128).transpose(0, 3, 2, 1, 4))
    wur = np.ascontiguousarray(
        _bf16(mlp_up).reshape(2, HK, 128, IK, 128).transpose(0, 3, 2, 1, 4))
    wdr = np.ascontiguousarray(_bf16(mlp_down).reshape(2, IK, 128, HK, 128))
    cwf = np.asarray(conv_w, dtype=np.float32)            # [2, H, KC]

    def fm_pack(cols_bf16, Cx):
        """[n, H] bf16 -> [128, HK, Cx] zero-padded feature-major."""
        n = cols_bf16.shape[0]
        arr = np.zeros((H, Cx), dtype=ml_dtypes.bfloat16)
        arr[:, :n] = cols_bf16.T
        return np.ascontiguousarray(arr.reshape(HK, 128, Cx).transpose(1, 0, 2))

    def bcast_row(vals, Cx):
        v = np.zeros(Cx, dtype=np.float32)
        v[: len(vals)] = vals
        return np.ascontiguousarray(
            np.broadcast_to(v[None, :], (128, Cx))).astype(ml_dtypes.bfloat16)

    in_maps = []
    for i in range(NCORES):
        im = {"wgr": wgr, "wur": wur, "wdr": wdr}
        for e in range(2):
            lst = lists[e][i]
            im[f"xg{e}"] = fm_pack(xflat_b[lst], Cs[e])
            im[f"cgb{e}"] = bcast_row(ce[lst, e], Cs[e])
        for e in range(2):
            lst = lists[2 + e][i]
            Cx = Cs[2 + e]
            n = len(lst)
            s_in_seq = lst % S
            # taps pre-scaled by the conv weight (w[h,j] folded into the
            # gathered activations): xq[j, c] = w[h,j]*x[lst[c]+j-3, h],
            # zero at sequence starts
            xc = np.zeros((128, HK, KC, Cx), dtype=ml_dtypes.bfloat16)
            for j in range(KC):
                src = lst + j - (KC - 1)
                valid = (s_in_seq + j - (KC - 1)) >= 0
                cols = np.where(valid[:, None],
                                xflat[src * valid] * cwf[e, :, j][None, :], 0)
                xc[:, :, j, :n] = _bf16(cols).T.reshape(
                    HK, 128, n).transpose(1, 0, 2)
            im[f"xc{e}"] = np.ascontiguousarray(xc)
            im[f"cgc{e}"] = bcast_row(ce[lst, 2 + e], Cx)
        in_maps.append(im)
    return in_maps


def assemble(results):
    lists = _ROUTE["lists"]
    out = np.zeros((B * S, H), dtype=np.float32)
    keys = ["yf0", "yf1", "yc0", "yc1"]
    for i, r in enumerate(results):
        for e in range(4):
            lst = lists[e][i]
            n = len(lst)
            yv = np.asarray(r[keys[e]], dtype=np.float32)  # [H, C_e]
            out[lst] += yv[:, :n].T
    return out.reshape(B, S, H)


def kernel(x, top_k_indices, norm_weights, mlp_gate, mlp_up, mlp_down, conv_w):
    in_maps = build_in_maps(
        x, top_k_indices, norm_weights, mlp_gate, mlp_up, mlp_down, conv_w
    )
    nc = build_nc()
    res = run_bass_kernel_spmd(nc, in_maps, core_ids=list(range(NCORES)))
    return assemble(res.results)


# revision 74
# speedup vs baseline: 1.1780x; 1.1780x over previous
"""MixedExpertLayer Trainium2 kernel, v11: fully routed, globally balanced MoE.

ALL four experts are routed on the host. For each expert the tokens with
nonzero combined weight c_e = sum_k w_k*[idx_k==e] (~43.75% of tokens) are
gathered into a compacted feature-major stream and split EVENLY across the 8
cores (no core affinity), so each core processes ~900 tokens per expert.

MLP experts 0,1 (per core, per expert, windows of <=512 tokens):
  gate/up: psum[i-tile, tok] = sum_h W[h,i]^T x[h, tok]   (feature-major)
  A = silu(g)*u on ACT+DVE
  down (feature-major): psum[h-tile, tok] = sum_i Wd[i,h]^T A[i, tok]
  scaled by c_e via one DVE mul with a host-broadcast coefficient row.

Conv experts 2,3: the host gathers FOUR tap-shifted copies of the selected
tokens (x[t-3+j] for j=0..3, zero at sequence starts), so the depthwise conv
becomes 4 accumulated diag-matrix matmuls on the PE over compacted columns
(diag(w_j) built on-device from an identity via ACT per-partition scaling),
then ACT silu and a DVE mul by the broadcast c_e row.

All outputs are compacted feature-major [H, C_e]; the host transposes and
scatter-adds the four streams into the zero-initialized result (fp32).

Conv ops are fed through a drain queue that interleaves them between MLP
matmul groups so no in-order engine stream is head-of-line blocked.
"""

import numpy as np
import ml_dtypes

import concourse.bass as bass
import concourse.mybir as mybir
import concourse.tile as tile
from concourse.bass_utils import run_bass_kernel_spmd

B, S, H, I, KTOP, KC = 4, 4096, 1024, 2048, 2, 4
NCORES = 8
T = (B * S) // NCORES          # 2048 tokens per core
TCH = 512                      # token chunk (matmul N / PSUM bank)
HK = H // 128                  # 8 h-chunks
IK = I // 128                  # 16 i-chunks
BF16 = mybir.dt.bfloat16
F32 = mybir.dt.float32
AF = mybir.ActivationFunctionType
MUL = mybir.AluOpType.mult
ADD = mybir.AluOpType.add

# routing state set by build_in_maps: per-expert device capacities and
# per-(expert, core) global token lists
_ROUTE = {"C": [1024, 1024, 1024, 1024], "lists": None}


def legalize_waits(nc):
    """This walrus build encodes exactly one sync-wait per instruction
    (single NEURON_ISA_TPB_EVENTS slot); Tile emits up to 3 plus a multi-wait
    tail Drain. Split extra waits onto wait-only EventSemaphore carriers
    inserted immediately before the instruction (same engine, same position,
    so no reordering and no deadlock risk)."""
    f = nc.m.functions[0]
    for blk in f.blocks:
        new = []
        for ins in list(blk.instructions):
            si = ins.sync_info
            if si is not None and si.on_wait and len(si.on_wait) > 1:
                best, order = {}, []
                for w in si.on_wait:
                    k = (w.sync_type, w.id, w.wait_mode)
                    if k not in best:
                        best[k] = w
                        order.append(k)
                    elif (w.wait_value or 0) > (best[k].wait_value or 0):
                        best[k] = w
                waits = [best[k] for k in order]
                for j, w in enumerate(waits[:-1]):
                    ev = mybir.InstEventSemaphore(
                        name=f"{ins.name}-lw{j}", engine=ins.engine, ins=[], outs=[],
                    )
                    ev.sync_info = mybir.SyncInfo(on_wait=[w], on_update=[])
                    new.append(ev)
                si.on_wait = [waits[-1]]
                ins.sync_info = si
            new.append(ins)
        blk.instructions = new
    return nc


def _windows(Ce):
    """Chunk windows (w0, n) covering Ce tokens in <=TCH pieces."""
    w, out = 0, []
    while w < Ce:
        n = min(TCH, Ce - w)
        out.append((w, n))
        w += n
    return out


def build_nc():
    C0, C1, C2, C3 = _ROUTE["C"]
    nc = bass.Bass(num_devices=NCORES)
    xg0 = nc.dram_tensor("xg0", [128, HK, C0], BF16, kind="ExternalInput")
    xg1 = nc.dram_tensor("xg1", [128, HK, C1], BF16, kind="ExternalInput")
    # xc streams are pre-scaled by the conv tap weight on the host
    # (w[h,j] folded into the gathered activations), so the depthwise conv
    # reduces to three 2-operand adds per hk slab on the DVE
    xc0 = nc.dram_tensor("xc0", [128, HK, KC, C2], BF16, kind="ExternalInput")
    xc1 = nc.dram_tensor("xc1", [128, HK, KC, C3], BF16, kind="ExternalInput")
    wgr = nc.dram_tensor("wgr", [2, IK, 128, HK, 128], BF16, kind="ExternalInput")
    wur = nc.dram_tensor("wur", [2, IK, 128, HK, 128], BF16, kind="ExternalInput")
    wdr = nc.dram_tensor("wdr", [2, IK, 128, HK, 128], BF16, kind="ExternalInput")
    cgb0 = nc.dram_tensor("cgb0", [128, C0], BF16, kind="ExternalInput")
    cgb1 = nc.dram_tensor("cgb1", [128, C1], BF16, kind="ExternalInput")
    cgc0 = nc.dram_tensor("cgc0", [128, C2], BF16, kind="ExternalInput")
    cgc1 = nc.dram_tensor("cgc1", [128, C3], BF16, kind="ExternalInput")
    yf0 = nc.dram_tensor("yf0", [H, C0], BF16, kind="ExternalOutput")
    yf1 = nc.dram_tensor("yf1", [H, C1], BF16, kind="ExternalOutput")
    yc0 = nc.dram_tensor("yc0", [H, C2], BF16, kind="ExternalOutput")
    yc1 = nc.dram_tensor("yc1", [H, C3], BF16, kind="ExternalOutput")

    yf_t = [y.rearrange("(o p) t -> p o t", p=128) for y in (yf0, yf1)]
    yc_t = [y.rearrange("(o p) t -> p o t", p=128) for y in (yc0, yc1)]
    xg_d = [xg0, xg1]
    xc_d = [xc0, xc1]
    cgb_d = [cgb0, cgb1]
    cgc_d = [cgc0, cgc1]
    Cconv = [C2, C3]

    phases = [(0, w0, n) for (w0, n) in _windows(C0)] + \
             [(1, w0, n) for (w0, n) in _windows(C1)]

    with tile.TileContext(nc) as tc:
        with (
            tc.tile_pool(name="singles", bufs=1) as singles,
            tc.tile_pool(name="wpool", bufs=17) as wpool,
            tc.tile_pool(name="wdpool", bufs=17) as wdpool,
            tc.tile_pool(name="apool", bufs=2) as apool,
            tc.tile_pool(name="xcpool", bufs=3) as xcpool,
            tc.tile_pool(name="spool", bufs=3) as spool,
            tc.tile_pool(name="sgp", bufs=2) as sgpool,
            tc.tile_pool(name="ytp", bufs=3) as ytpool,
            tc.tile_pool(name="ps", bufs=3, space="PSUM") as ps,
            tc.tile_pool(name="pd", bufs=2, space="PSUM") as pd,
        ):
            # ---- PE clock warmup: dummy matmuls with no DMA deps run while
            # the first input tiles stream in, so the DVFS ramp (0.65 ->
            # 2.4GHz over ~3us of continuous execution) finishes before the
            # real matmuls start ----
            scr = singles.tile([128, TCH], BF16)
            nc.gpsimd.memset(scr, 0.0)
            pw = pd.tile([128, TCH], F32, tag="pd", name="pwarm")
            for r in range(28):
                nc.tensor.matmul(pw, scr[:, 0:128], scr,
                                 start=(r == 0), stop=(r == 27))

            # ---- phase-0-critical DMAs first: xg0, then small state ----
            xg_sb = [singles.tile([128, HK, Cx], BF16, name=f"xg{i}")
                     for i, Cx in enumerate((C0, C1))]

            def xg_load(e, split=0):
                Cx = xg_sb[e].shape[-1]
                for hk in range(HK):
                    if hk < split:
                        h2 = Cx // 2
                        nc.sync.dma_start(
                            xg_sb[e][:, hk, 0:h2], xg_d[e][:, hk, 0:h2])
                        nc.sync.dma_start(
                            xg_sb[e][:, hk, h2:Cx], xg_d[e][:, hk, h2:Cx])
                    else:
                        nc.sync.dma_start(xg_sb[e][:, hk], xg_d[e][:, hk])

            xg_load(0)
            cgb_sb = [singles.tile([128, Cx], BF16, name=f"cgb{i}")
                      for i, Cx in enumerate((C0, C1))]
            cgc_sb = [singles.tile([128, Cx], BF16, name=f"cgc{i}")
                      for i, Cx in enumerate((C2, C3))]

            # ---- conv experts: drain-queue units of (e, hk) ----
            xc_tiles = {}

            def xc_fetch(u):
                if u >= 2 * HK:
                    return
                e, hk = divmod(u, HK)
                xct = xcpool.tile([128, KC, Cconv[e]], BF16, tag="xc",
                                  name="xct")
                for j in range(KC):
                    nc.sync.dma_start(xct[:, j, :], xc_d[e][:, hk, j, :])
                xc_tiles[u] = xct

            CCMAX = max(C2, C3)

            def conv_unit(u):
                e, hk = divmod(u, HK)
                Cc = Cconv[e]
                xct = xc_tiles.pop(u)
                acc = [None]

                # adds/mul on DVE: Pool is idle but ~3.5x slower per op, and
                # its lag delays xct buffer recycling, which blocks the next
                # xc_fetch in the in-order SP queue and starves the weight
                # stream behind it (measured +26us)
                def add01():
                    acc[0] = spool.tile([128, CCMAX], BF16, tag="sc",
                                        name="cacc")
                    nc.vector.tensor_add(
                        acc[0][:, 0:Cc], xct[:, 0, :], xct[:, 1, :])

                def addj(j):
                    def op():
                        nc.vector.tensor_add(
                            acc[0][:, 0:Cc], acc[0][:, 0:Cc], xct[:, j, :])
                    return op

                def silu():
                    nc.scalar.activation(
                        out=acc[0][:, 0:Cc], in_=acc[0][:, 0:Cc], func=AF.Silu)

                def mul():
                    nc.vector.tensor_mul(
                        acc[0][:, 0:Cc], acc[0][:, 0:Cc], cgc_sb[e])

                def store():
                    nc.sync.dma_start(yc_t[e][:, hk, :], acc[0][:, 0:Cc])

                return [add01, addj(2), addj(3), silu, mul, store,
                        lambda: xc_fetch(u + 3)]

            # (gate_slot, fn): fn runs no earlier than slot gate_slot, so the
            # heavy conv-input DMAs don't steal queues from phase-0 weights
            pending = [
                (3, lambda: nc.sync.dma_start(cgb_sb[0], cgb_d[0][:])),
                (3, lambda: nc.sync.dma_start(cgb_sb[1], cgb_d[1][:])),
                (5, lambda: nc.sync.dma_start(cgc_sb[0], cgc_d[0][:])),
                (5, lambda: nc.sync.dma_start(cgc_sb[1], cgc_d[1][:])),
                (7, lambda: xc_fetch(0)),
                (9, lambda: xc_fetch(1)),
                (11, lambda: xc_fetch(2)),
            ]
            conv_left = list(range(2 * HK))

            # drain only in gate/up slots: ops drained during the down phase
            # land on the DVE right before the NEXT phase's A-muls (which
            # gate PSUM recycling) and stall the PE at phase boundaries
            nslots = sum(IK for _ in phases)
            dn = max(2, -(-(len(pending) + 2 * HK * 7) // max(nslots - 13, 1)))

            slot_idx = [0]

            def drain(k):
                slot_idx[0] += 1
                for _ in range(k):
                    if not pending:
                        if not conv_left or slot_idx[0] <= 13:
                            return
                        pending.extend(
                            (0, f) for f in conv_unit(conv_left.pop(0)))
                    gate, fn = pending[0]
                    if slot_idx[0] < gate:
                        return
                    pending.pop(0)
                    fn()

            wds_by_e = {}
            wgu_by_e = {}
            for pi, (e, w0, nw) in enumerate(phases):
                # ---- gate/up -> A (feature-major [I, nw]) ----
                a_sb = apool.tile([128, IK, TCH], BF16, tag="a")
                wload = e not in wgu_by_e
                if wload:
                    wgu_by_e[e] = []
                need_wd = e not in wds_by_e
                if need_wd:
                    wds_by_e[e] = []
                for i in range(IK):
                    if wload:
                        # gate/up tiles stay resident across the expert's
                        # windows - loaded only on its first phase
                        wgt = wpool.tile([128, HK, 128], BF16, tag="wg")
                        wut = wpool.tile([128, HK, 128], BF16, tag="wu")
                        wgu_by_e[e].append((wgt, wut))
                        if pi == 0 and i < 3:
                            # split the very first weight tiles across two
                            # queues each: their latency gates the PE rampup
                            nc.sync.dma_start(wgt[:, 0:4], wgr[e, i, :, 0:4])
                            nc.sync.dma_start(wgt[:, 4:8], wgr[e, i, :, 4:8])
                            nc.sync.dma_start(wut[:, 0:4], wur[e, i, :, 0:4])
                            nc.sync.dma_start(wut[:, 4:8], wur[e, i, :, 4:8])
                        else:
                            nc.sync.dma_start(wgt, wgr[e, i])
                            nc.sync.dma_start(wut, wur[e, i])
                    else:
                        wgt, wut = wgu_by_e[e][i]
                    psg = ps.tile([128, TCH], F32, tag="pg")
                    psu = ps.tile([128, TCH], F32, tag="pu")
                    for kc in range(HK):
                        nc.tensor.matmul(
                            psg[:, 0:nw], wgt[:, kc, :],
                            xg_sb[e][:, kc, w0 : w0 + nw],
                            start=(kc == 0), stop=(kc == HK - 1))
                    for kc in range(HK):
                        nc.tensor.matmul(
                            psu[:, 0:nw], wut[:, kc, :],
                            xg_sb[e][:, kc, w0 : w0 + nw],
                            start=(kc == 0), stop=(kc == HK - 1))
                    sg = sgpool.tile([128, TCH], F32, tag="sg")
                    nc.scalar.activation(
                        out=sg[:, 0:nw], in_=psg[:, 0:nw], func=AF.Silu)
                    nc.vector.tensor_mul(
                        a_sb[:, i, 0:nw], sg[:, 0:nw], psu[:, 0:nw])
                    # mid-phase prefetch of down weights (shared across the
                    # expert's windows) and the next xg stream, spread over
                    # several iterations - emitted as one burst, the serial
                    # SP descriptor-gen (~0.9us per dma_start) starves the
                    # next gate/up weight tiles and stalls the PE
                    if need_wd and 8 <= i < 12:
                        for kc in range(4 * (i - 8), 4 * (i - 7)):
                            wdt = wdpool.tile([128, HK, 128], BF16,
                                              tag="wd", name="wdt")
                            nc.sync.dma_start(wdt, wdr[e, kc])
                            wds_by_e[e].append(wdt)
                    if pi == 0 and i in (12, 13):
                        for hk in range(4 * (i - 12), 4 * (i - 11)):
                            nc.sync.dma_start(xg_sb[1][:, hk], xg_d[1][:, hk])
                    drain(dn)

                # ---- down, feature-major: psum[h-tile, tok] ----
                wds = wds_by_e[e]
                for hb in range(HK):
                    psd = pd.tile([128, TCH], F32, tag="pd")
                    for kc in range(IK):
                        nc.tensor.matmul(
                            psd[:, 0:nw], wds[kc][:, hb, :],
                            a_sb[:, kc, 0:nw],
                            start=(kc == 0), stop=(kc == IK - 1))
                    yt = ytpool.tile([128, TCH], BF16, tag="yt")
                    nc.vector.tensor_mul(
                        yt[:, 0:nw], psd[:, 0:nw],
                        cgb_sb[e][:, w0 : w0 + nw])
                    if pi == len(phases) - 1:
                        # last phase: split the store for lower tail latency
                        h2 = nw // 2
                        nc.sync.dma_start(
                            yf_t[e][:, hb, w0 : w0 + h2], yt[:, 0:h2])
                        nc.sync.dma_start(
                            yf_t[e][:, hb, w0 + h2 : w0 + nw], yt[:, h2:nw])
                    else:
                        nc.sync.dma_start(
                            yf_t[e][:, hb, w0 : w0 + nw], yt[:, 0:nw])
            # flush any remaining conv work
            while pending or conv_left:
                drain(16)
    return legalize_waits(nc)


def _bf16(a):
    return np.asarray(a).astype(ml_dtypes.bfloat16)


def build_in_maps(x, top_k_indices, norm_weights, mlp_gate, mlp_up, mlp_down, conv_w):
    NT = B * S
    xflat = np.asarray(x, dtype=np.float32).reshape(NT, H)
    xflat_b = _bf16(xflat)
    idxflat = np.asarray(top_k_indices).reshape(NT, KTOP)
    nwflat = np.asarray(norm_weights, dtype=np.float32).reshape(NT, KTOP)

    # combined per-expert coefficients, global
    ce = np.zeros((NT, 4), dtype=np.float32)
    rows = np.arange(NT)
    for k in range(KTOP):
        np.add.at(ce, (rows, idxflat[:, k]), nwflat[:, k])

    # globally balanced routing: split every expert's token list evenly
    lists, Cs = [], []
    for e in range(4):
        glst = np.nonzero(ce[:, e] != 0.0)[0]
        parts = np.array_split(glst, NCORES)
        lists.append(parts)
        Cs.append(max(1, max(len(p) for p in parts)))
    _ROUTE["C"] = Cs
    _ROUTE["lists"] = lists

    # weights, repacked so every DMA tile is contiguous per partition
    wgr = np.ascontiguousarray(
        _bf16(mlp_gate).reshape(2, HK, 128, IK, 128).transpose(0, 3, 2, 1, 4))
    wur = np.ascontiguousarray(
        _bf16(mlp_up).reshape(2, HK, 128, IK, 128).transpose(0, 3, 2, 1, 4))
    wdr = np.ascontiguousarray(_bf16(mlp_down).reshape(2, IK, 128, HK, 128))
    cwf = np.asarray(conv_w, dtype=np.float32)            # [2, H, KC]

    def fm_pack(cols_bf16, Cx):
        """[n, H] bf16 -> [128, HK, Cx] zero-padded feature-major."""
        n = cols_bf16.shape[0]
        arr = np.zeros((H, Cx), dtype=ml_dtypes.bfloat16)
        arr[:, :n] = cols_bf16.T
        return np.ascontiguousarray(arr.reshape(HK, 128, Cx).transpose(1, 0, 2))

    def bcast_row(vals, Cx):
        v = np.zeros(Cx, dtype=np.float32)
        v[: len(vals)] = vals
        return np.ascontiguousarray(
            np.broadcast_to(v[None, :], (128, Cx))).astype(ml_dtypes.bfloat16)

    in_maps = []
    for i in range(NCORES):
        im = {"wgr": wgr, "wur": wur, "wdr": wdr}
        for e in range(2):
            lst = lists[e][i]
            im[f"xg{e}"] = fm_pack(xflat_b[lst], Cs[e])
            im[f"cgb{e}"] = bcast_row(ce[lst, e], Cs[e])
        for e in range(2):
            lst = lists[2 + e][i]
            Cx = Cs[2 + e]
            n = len(lst)
            s_in_seq = lst % S
            # taps pre-scaled by the conv weight (w[h,j] folded into the
            # gathered activations): xq[j, c] = w[h,j]*x[lst[c]+j-3, h],
            # zero at sequence starts
            xc = np.zeros((128, HK, KC, Cx), dtype=ml_dtypes.bfloat16)
            for j in range(KC):
                src = lst + j - (KC - 1)
                valid = (s_in_seq + j - (KC - 1)) >= 0
                cols = np.where(valid[:, None],
                                xflat[src * valid] * cwf[e, :, j][None, :], 0)
                xc[:, :, j, :n] = _bf16(cols).T.reshape(
                    HK, 128, n).transpose(1, 0, 2)
            im[f"xc{e}"] = np.ascontiguousarray(xc)
            im[f"cgc{e}"] = bcast_row(ce[lst, 2 + e], Cx)
        in_maps.append(im)
    return in_maps


def assemble(results):
    lists = _ROUTE["lists"]
    out = np.zeros((B * S, H), dtype=np.float32)
    keys = ["yf0", "yf1", "yc0", "yc1"]
    for i, r in enumerate(results):
        for e in range(4):
            lst = lists[e][i]
            n = len(lst)
            yv = np.asarray(r[keys[e]], dtype=np.float32)  # [H, C_e]
            out[lst] += yv[:, :n].T
    return out.reshape(B, S, H)


def kernel(x, top_k_indices, norm_weights, mlp_gate, mlp_up, mlp_down, conv_w):
    in_maps = build_in_maps(
        x, top_k_indices, norm_weights, mlp_gate, mlp_up, mlp_down, conv_w
    )
    nc = build_nc()
    res = run_bass_kernel_spmd(nc, in_maps, core_ids=list(range(NCORES)))
    return assemble(res.results)
